# revision 2
# baseline (speedup 1.0000x reference)
"""Causal self-attention on 8 TRN2 NeuronCores.

Problem: x[2,2048,1024], wq/wk/wv/wo[1024,1024] (nn.Linear convention,
out = y @ W.T), H=16 heads, D=64, causal softmax, f32 in/out.

Sharding: tensor-parallel over heads x data-parallel over batch.
Core i handles batch b=i//4 and head group g=i%4 (4 heads each).
Each core returns a partial output projection outT[b] and the host
sums the 4 partials per batch.

v2 design notes (from the v1 trace: PE busy 98% of a 323us span,
fp32_mode=HIGH matmuls with serial internal weight loads, and a HAM
clock re-throttle to 1.2GHz for 202us of the run):

- Everything the PE touches is bf16 (host-side cast): bf16 matmuls
  stream 1 column/cycle at any N, get fast weight loads (FWL), and
  allow the standalone LDWEIGHTS to overlap in-flight matmuls via the
  PE's reorder window. Inputs DMA at half the bytes too.
- Scores for a head PAIR run concurrently in the PE array via row
  tiling: head 2p lives on SBUF partitions 0-63, head 2p+1 on 64-127,
  so the two 64-row stationaries occupy disjoint row groups
  (tile_position (0,0) / (64,0)) and the two matmuls overlap to ~1x
  the single-matmul duration, doubling effective utilization of the
  64-deep contraction.
- Attention is span-outer (512 queries per span), pairs interleaved by
  span, with a single wide exp per (pair, span, key-chunk) over both
  heads' score halves (a 2-block strided AP over the 2-bank PSUM mg
  tile). Causal masking is a multiplicative bf16 triangular mask on
  the diagonal 128-strips after exp. Softmax row sums ride a
  ones-column folded into V's stationary (65-wide PV matmuls).
- Normalization per (head, span): DVE copies PV out of PSUM (frees
  the bank fast), reciprocal of the sums row, one DMA roundtrip to
  broadcast 1/sum across 64 partitions (stride-0 partition read), and
  a gpsimd multiply into the bf16 y tile.
- The output projection trails attention by one span so its matmuls
  fill PE gaps while the Act engine (the attention-phase bottleneck,
  ~1 exp elem/cycle/partition) streams, and its stores overlap.
"""

import sys

for _p in ("/opt/trn_rl_repo", "/root/.axon_site"):
    if _p not in sys.path:
        sys.path.insert(0, _p)

import ml_dtypes
import numpy as np

import concourse.bass as bass
import concourse.mybir as mybir
import concourse.tile as tile
from concourse import bacc
from concourse.bass_utils import run_bass_kernel_spmd

B, T, C, H = 2, 2048, 1024, 16
DH = C // H            # 64 head dim
HG = 4                 # heads per core
GW = HG * DH           # 256 features per head group
NS = T // 512          # 4 query spans
KC = C // 128          # 8 contraction chunks over C
SCALE = 1.0 / float(np.sqrt(DH))
N_CORES = 8

F32 = mybir.dt.float32
BF16 = mybir.dt.bfloat16
EXP = mybir.ActivationFunctionType.Exp
COPY = mybir.ActivationFunctionType.Copy


def build_nc():
    nc = bacc.Bacc("TRN2", target_bir_lowering=False, debug=False,
                   num_devices=N_CORES)
    xT = nc.declare_dram_parameter("xT", [C, T], BF16, isOutput=False)
    wqT = nc.declare_dram_parameter("wqT", [C, GW], BF16, isOutput=False)
    wkT = nc.declare_dram_parameter("wkT", [C, GW], BF16, isOutput=False)
    wvT = nc.declare_dram_parameter("wvT", [C, GW], BF16, isOutput=False)
    woT = nc.declare_dram_parameter("woT", [GW, C], BF16, isOutput=False)
    outT = nc.declare_dram_parameter("outT", [C, T], F32, isOutput=True)
    # per (pair, head-in-pair, span): 1/rowsum scratch for the
    # partition-broadcast roundtrip
    r_dram = nc.dram_tensor("r_scratch", [2, 2, NS, 512], F32)

    with tile.TileContext(nc) as tc:
        with tc.tile_pool(name="pers", bufs=1) as pers:
            # ---- persistent SBUF; DMAs issued in consumption order ----
            wk_t = [pers.tile([128, GW], BF16, tag=f"wk{i}", name=f"wk{i}")
                    for i in range(KC)]
            for i in range(KC):
                nc.sync.dma_start(out=wk_t[i], in_=wkT[i * 128:(i + 1) * 128, :])
            xts = [pers.tile([128, T], BF16, tag=f"xT{i}", name=f"xT{i}")
                   for i in range(KC)]
            wq_t = [pers.tile([128, GW], BF16, tag=f"wq{i}", name=f"wq{i}")
                    for i in range(KC)]
            wv_t = [pers.tile([128, GW], BF16, tag=f"wv{i}", name=f"wv{i}")
                    for i in range(KC)]
            for s in range(NS):          # span-sized sub-loads
                for i in range(KC):
                    nc.sync.dma_start(
                        out=xts[i][:, s * 512:(s + 1) * 512],
                        in_=xT[i * 128:(i + 1) * 128, s * 512:(s + 1) * 512])
                if s == 0:
                    for i in range(KC):
                        nc.sync.dma_start(
                            out=wq_t[i], in_=wqT[i * 128:(i + 1) * 128, :])
                if s == 1:
                    for i in range(KC):
                        nc.sync.dma_start(
                            out=wv_t[i], in_=wvT[i * 128:(i + 1) * 128, :])
            wo_t = [pers.tile([128, C], BF16, tag=f"wo{j}", name=f"wo{j}")
                    for j in range(2)]
            for j in range(2):
                nc.sync.dma_start(out=wo_t[j], in_=woT[j * 128:(j + 1) * 128, :])

            # proj outputs / attention outputs, bf16, feature-major
            qts = [pers.tile([128, T], BF16, tag=f"qT{m}", name=f"qT{m}")
                   for m in range(2)]
            kts = [pers.tile([128, T], BF16, tag=f"kT{m}", name=f"kT{m}")
                   for m in range(2)]
            yts = [pers.tile([128, T], BF16, tag=f"yT{m}", name=f"yT{m}")
                   for m in range(2)]

            # bf16 triangular mask for the diagonal 128x128 strip of
            # P^T: keep (1) where col >= row i.e. q >= k, else 0
            trim = pers.tile([128, 128], BF16, tag="trim", name="trim")
            nc.gpsimd.memset(trim, 1.0)
            nc.gpsimd.affine_select(
                out=trim, in_=trim, compare_op=mybir.AluOpType.is_ge,
                fill=0.0, base=0, pattern=[[1, 128]], channel_multiplier=-1)
            # ones [128, 4] in bf16 for V's ones-columns
            ones4 = pers.tile([128, 4], BF16, tag="ones4", name="ones4")
            for j in range(4):
                nc.scalar.activation(
                    out=ones4[:, j:j + 1],
                    in_=nc.const_aps.tensor(1.0, [128, 1]), func=COPY)

            # V in natural [t, d] layout, 65-wide per head (64 v + one)
            vts = [pers.tile([128, HG * 65], BF16, tag=f"V{tb}", name=f"V{tb}")
                   for tb in range(T // 128)]

            # ---- phase 1: projections (all-bf16, PE-dense) ----
            with tc.tile_pool(name="pp1", bufs=4, space="PSUM") as pp1, \
                 tc.tile_pool(name="pp2", bufs=2, space="PSUM") as pp2:
                # k then q for m=0 (pair0 consumes them first), then V,
                # then k,q for m=1
                def proj_block(wt, dest, m):
                    for s in range(NS):
                        ps = pp1.tile([128, 512], F32, tag="projps",
                                      name="projps")
                        for k in range(KC):
                            nc.tensor.matmul(
                                ps,
                                wt[k][:, m * 128:(m + 1) * 128],
                                xts[k][:, s * 512:(s + 1) * 512],
                                start=(k == 0), stop=(k == KC - 1))
                        nc.vector.tensor_copy(
                            out=dest[m][:, s * 512:(s + 1) * 512], in_=ps)

                proj_block(wk_t, kts, 0)
                proj_block(wq_t, qts, 0)
                for tb in range(T // 128):
                    vps = pp2.tile([128, GW], F32, tag="vps", name="vps")
                    for k in range(KC):
                        nc.tensor.matmul(
                            vps, xts[k][:, tb * 128:(tb + 1) * 128], wv_t[k],
                            start=(k == 0), stop=(k == KC - 1))
                    vt = vts[tb]
                    nc.vector.tensor_copy(
                        out=vt.rearrange("p (h c) -> p h c", c=65)[:, :, 0:64],
                        in_=vps.rearrange("p (h c) -> p h c", c=64))
                    nc.vector.tensor_copy(
                        out=vt.rearrange("p (h c) -> p h c", c=65)[:, :, 64],
                        in_=ones4)
                proj_block(wk_t, kts, 1)
                proj_block(wq_t, qts, 1)

            # ---- phase 2: attention, span-outer, head-pair row-tiled;
            # out-projection trails by one span ----
            with tc.tile_pool(name="mgs", bufs=2, space="PSUM") as mgs, \
                 tc.tile_pool(name="pvs", bufs=1, space="PSUM") as pvs, \
                 tc.tile_pool(name="ops", bufs=2, space="PSUM") as ops, \
                 tc.tile_pool(name="ptp", bufs=3) as ptp, \
                 tc.tile_pool(name="rp", bufs=4) as rp, \
                 tc.tile_pool(name="ost", bufs=4) as ost:

                def outproj(s):
                    for m in range(8):
                        op = ops.tile([128, 512], F32, tag="op", name="op")
                        for j in range(2):
                            nc.tensor.matmul(
                                op,
                                wo_t[j][:, m * 128:(m + 1) * 128],
                                yts[j][:, s * 512:(s + 1) * 512],
                                start=(j == 0), stop=(j == 1))
                        ot = ost.tile([128, 512], F32, tag="ot", name="ot")
                        nc.vector.tensor_copy(out=ot, in_=op)
                        nc.sync.dma_start(
                            out=outT[m * 128:(m + 1) * 128,
                                     s * 512:(s + 1) * 512],
                            in_=ot)

                for s in range(NS):
                    for p in range(2):
                        qt, kt, yt = qts[p], kts[p], yts[p]
                        pv = [pvs.tile([65, 512], F32, tag=f"pv{hl}",
                                       name=f"pv{hl}") for hl in range(2)]
                        for ki in range(4 * s + 4):
                            c0 = 128 * (ki - 4 * s) if ki >= 4 * s else 0
                            w = 512 - c0
                            q0 = s * 512 + c0
                            mg = mgs.tile([128, 1024], F32, tag="mg",
                                          name="mg")
                            # paired scores: head 2p rows 0-63 ->
                            # bank A cols, head 2p+1 rows 64-127 ->
                            # bank B cols; concurrent via row tiling
                            nc.tensor.matmul(
                                mg[:, c0:512],
                                kt[0:64, ki * 128:(ki + 1) * 128],
                                qt[0:64, q0:(s + 1) * 512],
                                start=True, stop=True)
                            nc.tensor.matmul(
                                mg[:, 512 + c0:1024],
                                kt[64:128, ki * 128:(ki + 1) * 128],
                                qt[64:128, q0:(s + 1) * 512],
                                start=True, stop=True)
                            # one exp over both heads' halves
                            pt = ptp.tile([128, 1024], BF16, tag="pt",
                                          name="pt")
                            mga = bass.AP(
                                tensor=mg.tensor, offset=mg.offset + c0,
                                ap=[list(mg.ap[0]), [512, 2], [1, w]])
                            pta = bass.AP(
                                tensor=pt.tensor, offset=pt.offset + c0,
                                ap=[list(pt.ap[0]), [512, 2], [1, w]])
                            nc.scalar.activation(
                                out=pta, in_=mga, func=EXP, scale=SCALE)
                            if ki >= 4 * s:
                                # causal mask on the diagonal strips
                                nc.vector.tensor_mul(
                                    out=pt[:, c0:c0 + 128],
                                    in0=pt[:, c0:c0 + 128], in1=trim)
                                nc.vector.tensor_mul(
                                    out=pt[:, 512 + c0:512 + c0 + 128],
                                    in0=pt[:, 512 + c0:512 + c0 + 128],
                                    in1=trim)
                            for hl in range(2):
                                h = 2 * p + hl
                                nc.tensor.matmul(
                                    pv[hl][:, c0:512],
                                    vts[ki][:, h * 65:(h + 1) * 65],
                                    pt[:, 512 * hl + c0:512 * hl + c0 + w],
                                    start=(ki == 0), stop=(ki == 4 * s + 3))
                        # finalize both heads of (pair p, span s)
                        for hl in range(2):
                            po = hl * 64
                            yv = rp.tile([65, 512], F32, tag=f"yv{hl}",
                                         name=f"yv{hl}")
                            nc.vector.tensor_copy(out=yv, in_=pv[hl][0:65, :])
                            rr = rp.tile([1, 512], F32, tag=f"rr{hl}",
                                         name=f"rr{hl}")
                            nc.vector.reciprocal(out=rr, in_=yv[64:65, :])
                            nc.gpsimd.dma_start(out=r_dram[p, hl, s, :],
                                                in_=rr)
                            rb = rp.tile([64, 512], F32, tag=f"rb{hl}",
                                         name=f"rb{hl}")
                            rsl = r_dram[p, hl, s, :]
                            nc.gpsimd.dma_start(
                                out=rb,
                                in_=bass.AP(tensor=rsl.tensor,
                                            offset=rsl.offset,
                                            ap=[[0, 64]] + list(rsl.ap)))
                            nc.gpsimd.tensor_mul(
                                out=yt[po:po + 64, s * 512:(s + 1) * 512],
                                in0=yv[0:64, :], in1=rb)
                    if s > 0:
                        outproj(s - 1)
                outproj(NS - 1)
    nc.compile()
    return nc


_NC_CACHE = None


def _get_nc():
    global _NC_CACHE
    if _NC_CACHE is None:
        _NC_CACHE = build_nc()
    return _NC_CACHE


def make_in_maps(x, wq, wk, wv, wo):
    bf = ml_dtypes.bfloat16
    x = np.asarray(x, dtype=np.float32)
    wq = np.asarray(wq, dtype=np.float32)
    wk = np.asarray(wk, dtype=np.float32)
    wv = np.asarray(wv, dtype=np.float32)
    wo = np.asarray(wo, dtype=np.float32)
    in_maps = []
    for core in range(N_CORES):
        b, g = core // HG, core % HG
        rows = slice(g * GW, (g + 1) * GW)
        in_maps.append({
            "xT": np.ascontiguousarray(x[b].T).astype(bf),
            "wqT": np.ascontiguousarray(wq[rows, :].T).astype(bf),
            "wkT": np.ascontiguousarray(wk[rows, :].T).astype(bf),
            "wvT": np.ascontiguousarray(wv[rows, :].T).astype(bf),
            "woT": np.ascontiguousarray(wo[:, rows].T).astype(bf),
        })
    return in_maps


def run(x, wq, wk, wv, wo, trace=False, tmpdir=None):
    nc = _get_nc()
    in_maps = make_in_maps(x, wq, wk, wv, wo)
    res = run_bass_kernel_spmd(nc, in_maps, core_ids=list(range(N_CORES)),
                               trace=trace, tmpdir=tmpdir)
    out = np.zeros((B, T, C), dtype=np.float32)
    for core in range(N_CORES):
        out[core // HG] += res.results[core]["outT"].T
    return out, res


def kernel(x, wq, wk, wv, wo):
    out, _ = run(x, wq, wk, wv, wo)
    return out
